# revision 1
# baseline (speedup 1.0000x reference)
"""Sliding-window causal self-attention (GQA + RoPE + QK-RMSNorm + ve-gate) on
8 Trainium2 NeuronCores.

Sharding: core c handles (batch b = c // 4, kv-head g = c % 4): data parallel
over batch x tensor parallel over the 4 KV head groups (4 query heads per
core). Each core computes its partial c_proj output; the all-reduce over the 4
head shards is a host-side sum.

Device design (per core):
  - x is fed transposed (xT: C x T) so all projections contract over the
    partition axis.
  - q, k are built transposed (qT/kT: head-dim x T); scores are computed
    TRANSPOSED (S^T: tk x tq) so softmax denominators come from a ones-matmul
    on the Tensor engine and P@V needs no transposes.
  - softmax skips max-subtraction: QK RMS-norm bounds |scores| <= 1.44*sqrt(128)
    so exp() cannot overflow in fp32. Masking is a -100 bias on the two
    triangular boundary blocks; masked weights underflow to 0.
  - k's rms-norm scale rides the per-partition `scale` operand of the Exp
    activation; q's rides the PSUM-evacuation multiply.
  - all matmuls run in float32r (full PE rate for moving dim >= 256,
    ~1.6e-4 matmul relerr vs fp32).
"""

import sys

sys.path.insert(0, "/opt/trn_rl_repo")

import numpy as np

B, T, C = 2, 2048, 2048
NH, NKV, HD = 16, 4, 128
GATE_CH = 12
HPC = NH // NKV          # q heads per core
TS = 512                 # token-slice width
NSL = T // TS            # 4 slices
NCK = C // 128           # 16 contraction chunks
TPS = TS // 128          # 4 token tiles per slice
NTT = T // 128           # 16 token tiles
EPS = 1e-6
NEG = -100.0

A_Q = 1.2 / np.sqrt(float(HD))   # rms-norm scale folded into q (incl 1/sqrt(HD))
A_K = 1.2                        # rms-norm scale folded into exp() scale arg
S_Q = float(1.0 / (HD * A_Q * A_Q))
B_Q = float(EPS / (A_Q * A_Q))
S_K = float(1.0 / (HD * A_K * A_K))
B_K = float(EPS / (A_K * A_K))

_compiled = {}


def _ktiles(m4, W):
    """k-tiles overlapping q-slice m4 with their valid tq-column extents.

    Returns list of (n, f0, f1, causal_block_col, edge_block_col); columns are
    relative to the slice (0..TS). First entry covers [0, TS) fully (it opens
    the PSUM accumulation group).
    """
    assert W % 128 == 0 and W >= 384
    out = []
    for n in range(0, TPS * m4 + TPS):
        f0 = max(0, 128 * n - TS * m4)
        f1 = min(TS, 128 * n + W + 128 - TS * m4)
        if f1 <= f0:
            continue
        causal = 128 * n >= TS * m4            # diagonal staircase inside tile
        edge = (128 * n + W + 128 - TS * m4) <= TS  # window lower edge inside
        cb = f0 if causal else None
        eb = (f1 - 128) if edge else None
        out.append((n, f0, f1, cb, eb))
    full = [e for e in out if e[1] == 0 and e[2] == TS]
    assert full, "need one full-extent tile to open the PSUM group"
    first = full[0]
    rest = [e for e in out if e[0] != first[0]]
    return [first] + rest


def _build(W):
    import concourse.bass as bass
    import concourse.tile as tile
    from concourse import bacc, mybir
    from concourse.masks import make_identity
    from contextlib import ExitStack

    f32 = mybir.dt.float32
    f32r = mybir.dt.float32r
    AF = mybir.ActivationFunctionType
    OP = mybir.AluOpType

    nc = bacc.Bacc(None, target_bir_lowering=False)

    xT = nc.dram_tensor("xT", [C, T], f32r, kind="ExternalInput")
    wq = nc.dram_tensor("wqT", [C, HPC * HD], f32r, kind="ExternalInput")
    wk = nc.dram_tensor("wkT", [C, HD], f32r, kind="ExternalInput")
    wv = nc.dram_tensor("wvT", [C, HD], f32r, kind="ExternalInput")
    wp = nc.dram_tensor("wpT", [HPC * HD, C], f32r, kind="ExternalInput")
    wgd = nc.dram_tensor("wg", [GATE_CH, 1], f32r, kind="ExternalInput")
    ccd = nc.dram_tensor("cc", [HD, T], f32, kind="ExternalInput")
    ssd = nc.dram_tensor("ss", [HD, T], f32, kind="ExternalInput")
    ved = nc.dram_tensor("ve", [T, HD], f32, kind="ExternalInput")
    btrid = nc.dram_tensor("btri", [128, 128], f32, kind="ExternalInput")
    etrid = nc.dram_tensor("etri", [128, 128], f32, kind="ExternalInput")
    outT = nc.dram_tensor("outT", [C, T], f32, kind="ExternalOutput")

    with tile.TileContext(nc) as tc, ExitStack() as ctx:
        res = ctx.enter_context(tc.tile_pool(name="res", bufs=1))
        xc_p = ctx.enter_context(tc.tile_pool(name="xc", bufs=1))
        tab_p = ctx.enter_context(tc.tile_pool(name="tab", bufs=1))
        work_p = ctx.enter_context(tc.tile_pool(name="work", bufs=2))
        sq_p = ctx.enter_context(tc.tile_pool(name="sq", bufs=3))
        bc_p = ctx.enter_context(tc.tile_pool(name="bc", bufs=2))
        qt_p = ctx.enter_context(tc.tile_pool(name="qt", bufs=2))
        es_p = ctx.enter_context(tc.tile_pool(name="es", bufs=4))
        yt_p = ctx.enter_context(tc.tile_pool(name="yt", bufs=1))
        ot_p = ctx.enter_context(tc.tile_pool(name="ot", bufs=3))
        row_p = ctx.enter_context(tc.tile_pool(name="rows", bufs=1))

        ps_qkv = ctx.enter_context(tc.tile_pool(name="ps_qkv", bufs=2, space="PSUM"))
        ps_s = ctx.enter_context(tc.tile_pool(name="ps_s", bufs=3, space="PSUM"))
        ps_row = ctx.enter_context(tc.tile_pool(name="ps_row", bufs=3, space="PSUM"))
        dram_p = ctx.enter_context(tc.tile_pool(name="dram", bufs=2, space="DRAM"))

        # resident tensors; weight loads split per chunk so the first QKV
        # matmuls can start as soon as their chunk lands (startup latency).
        wq_sb = res.tile([128, NCK, HPC * HD], f32r)
        wk_sb = res.tile([128, NCK, HD], f32r)
        wv_sb = res.tile([128, NCK, HD], f32r)
        wp_sb = res.tile([128, HPC, C], f32r)   # loaded later, before cproj(0)
        wg_sb = res.tile([GATE_CH, 1], f32r)
        nc.sync.dma_start(out=wg_sb, in_=wgd[:, :])
        btri_sb = res.tile([128, 128], f32)
        nc.sync.dma_start(out=btri_sb, in_=btrid[:, :])
        etri_sb = res.tile([128, 128], f32)
        nc.sync.dma_start(out=etri_sb, in_=etrid[:, :])
        ident = res.tile([128, 128], f32)
        make_identity(nc, ident)
        ones_f = res.tile([128, 1], f32)
        nc.vector.memset(ones_f, 1.0)
        ones_sb = ones_f.bitcast(f32r)
        bq_sb = res.tile([1, 1], f32)
        nc.vector.memset(bq_sb, B_Q)
        bk_sb = res.tile([128, 1], f32)
        nc.vector.memset(bk_sb, B_K)
        kT_sb = res.tile([128, T], f32r)        # rotated k, head-dim on partitions
        vn_sb = res.tile([128, NTT, HD], f32r)  # v natural, token tiles on partitions
        rnk_sb = res.tile([128, NTT], f32)      # per-k-tile rms-norm columns

        def rope_inplace(dst, cc_sl, ss_sl):
            """dst (128, TS) f32r holding pre-rotation values. In-place RoPE."""
            qsw = work_p.tile([128, TS], f32, tag="qsw")
            nc.sync.dma_start(out=qsw[0:64, :], in_=dst[64:128, :].bitcast(f32))
            nc.sync.dma_start(out=qsw[64:128, :], in_=dst[0:64, :].bitcast(f32))
            tmp = work_p.tile([128, TS], f32, tag="tmp")
            nc.gpsimd.tensor_mul(tmp, qsw, ss_sl)
            nc.vector.tensor_mul(dst, dst.bitcast(f32), cc_sl)
            nc.vector.tensor_add(dst, dst.bitcast(f32), tmp)

        for m4 in range(NSL):
            t0 = m4 * TS
            # ---- stream x slice + tables ----
            xc = []
            for c in range(NCK):
                xt = xc_p.tile([128, TS], f32r, tag=f"xc{c}")
                nc.sync.dma_start(out=xt, in_=xT[c * 128:(c + 1) * 128, t0:t0 + TS])
                xc.append(xt)
                if m4 == 0:
                    nc.sync.dma_start(out=wk_sb[:, c, :],
                                      in_=wk[c * 128:(c + 1) * 128, :])
            cc_sl = tab_p.tile([128, TS], f32, tag="cc")
            nc.sync.dma_start(out=cc_sl, in_=ccd[:, t0:t0 + TS])
            ss_sl = tab_p.tile([128, TS], f32, tag="ss")
            nc.sync.dma_start(out=ss_sl, in_=ssd[:, t0:t0 + TS])
            ve_sl = tab_p.tile([128, TPS, HD], f32, tag="ve")
            nc.sync.dma_start(
                out=ve_sl, in_=ved[t0:t0 + TS, :].rearrange("(tt p) h -> p tt h", p=128)
            )

            # ---- gate columns: 3*sigmoid(x[:, :12] @ wg) ----
            ps_g = ps_row.tile([1, TS], f32, tag="rows")
            nc.tensor.matmul(ps_g, wg_sb, xc[0][0:GATE_CH, :], start=True, stop=True)
            g_row = row_p.tile([1, TS], f32, tag="grow")
            nc.scalar.activation(g_row, ps_g, AF.Exp, scale=-1.0)
            nc.vector.tensor_scalar(out=g_row, in0=g_row, scalar1=1.0, scalar2=None,
                                    op0=OP.add)
            nc.vector.reciprocal(g_row, g_row)
            g_dr = dram_p.tile([TS], f32, tag="gdr")
            nc.sync.dma_start(out=g_dr, in_=g_row)
            gate_c = row_p.tile([128, TPS], f32, tag="gate")
            nc.sync.dma_start(
                out=gate_c,
                in_=bass.AP(tensor=g_dr.tensor, offset=g_dr.offset,
                            ap=[[1, 128], [128, TPS]]),
            )

            # ---- k projection + rms-norm cols + rope ----
            ps_k = ps_qkv.tile([128, TS], f32, tag="qkv")
            for c in range(NCK):
                nc.tensor.matmul(ps_k, wk_sb[:, c, :], xc[c],
                                 start=(c == 0), stop=(c == NCK - 1))
            sq_k = sq_p.tile([128, TS], f32r, tag="sq")
            nc.scalar.activation(sq_k, ps_k, AF.Square)
            ps_rk = ps_row.tile([1, TS], f32, tag="rows")
            nc.tensor.matmul(ps_rk, ones_sb, sq_k, start=True, stop=True)
            srk = row_p.tile([1, TS], f32, tag="srk")
            nc.scalar.activation(srk, ps_rk, AF.Ln, bias=bk_sb[0:1], scale=S_K)
            nc.scalar.activation(srk, srk, AF.Exp, scale=-0.5)
            k_dr = dram_p.tile([TS], f32, tag="kdr")
            nc.sync.dma_start(out=k_dr, in_=srk)
            nc.sync.dma_start(
                out=rnk_sb[:, m4 * TPS:(m4 + 1) * TPS],
                in_=bass.AP(tensor=k_dr.tensor, offset=k_dr.offset,
                            ap=[[1, 128], [128, TPS]]),
            )
            k_sl = kT_sb[:, t0:t0 + TS]
            nc.vector.tensor_copy(k_sl, ps_k)
            rope_inplace(k_sl, cc_sl, ss_sl)

            # ---- v projection + transpose to natural + gate-add ----
            if m4 == 0:
                for c in range(NCK):
                    nc.sync.dma_start(out=wv_sb[:, c, :],
                                      in_=wv[c * 128:(c + 1) * 128, :])
            ps_v = ps_qkv.tile([128, TS], f32, tag="qkv")
            for c in range(NCK):
                nc.tensor.matmul(ps_v, wv_sb[:, c, :], xc[c],
                                 start=(c == 0), stop=(c == NCK - 1))
            vT_s = work_p.tile([128, TS], f32, tag="qsw")
            nc.vector.tensor_copy(vT_s, ps_v)
            for tt in range(TPS):
                ps_t = ps_s.tile([128, TS], f32, tag="s")
                nc.tensor.transpose(ps_t[:, 0:128], vT_s[:, tt * 128:(tt + 1) * 128],
                                    ident)
                gtmp = work_p.tile([128, HD], f32, tag="gtmp")
                nc.vector.tensor_scalar(out=gtmp, in0=ve_sl[:, tt, :],
                                        scalar1=gate_c[:, tt:tt + 1], scalar2=3.0,
                                        op0=OP.mult, op1=OP.mult)
                nc.vector.tensor_add(vn_sb[:, m4 * TPS + tt, :], ps_t[:, 0:128], gtmp)

            # ---- q projections (4 heads) + rms-norm + rope ----
            if m4 == 0:
                for c in range(NCK):
                    nc.sync.dma_start(out=wq_sb[:, c, :],
                                      in_=wq[c * 128:(c + 1) * 128, :])
            qts = []
            for h in range(HPC):
                ps_q = ps_qkv.tile([128, TS], f32, tag="qkv")
                for c in range(NCK):
                    nc.tensor.matmul(ps_q, wq_sb[:, c, h * HD:(h + 1) * HD], xc[c],
                                     start=(c == 0), stop=(c == NCK - 1))
                sq_q = sq_p.tile([128, TS], f32r, tag="sq")
                nc.scalar.activation(sq_q, ps_q, AF.Square)
                ps_r = ps_row.tile([1, TS], f32, tag="rows")
                nc.tensor.matmul(ps_r, ones_sb, sq_q, start=True, stop=True)
                srow = row_p.tile([1, TS], f32, tag="srow")
                nc.scalar.activation(srow, ps_r, AF.Ln, bias=bq_sb, scale=S_Q)
                nc.scalar.activation(srow, srow, AF.Exp, scale=-0.5)
                rbc = bc_p.tile([128, TS], f32, tag="bc")
                nc.gpsimd.partition_broadcast(rbc, srow)
                qt = qt_p.tile([128, TS], f32r, tag=f"qt{h}")
                nc.vector.tensor_mul(qt, ps_q, rbc)
                rope_inplace(qt, cc_sl, ss_sl)
                qts.append(qt)

            # ---- attention (scores transposed: tk on partitions, tq free) ----
            tiles = _ktiles(m4, W)
            last = len(tiles) - 1
            yts = []
            for h in range(HPC):
                ps_out = ps_row.tile([128, TS], f32, tag="rows")
                ps_sum = ps_row.tile([1, TS], f32, tag="rows")
                for idx, (n, f0, f1, cb, eb) in enumerate(tiles):
                    pss = ps_s.tile([128, TS], f32, tag="s")
                    nc.tensor.matmul(pss[:, f0:f1], kT_sb[:, n * 128:(n + 1) * 128],
                                     qts[h][:, f0:f1], start=True, stop=True)
                    es = es_p.tile([128, TS], f32r, tag="es")
                    nc.scalar.activation(es[:, f0:f1], pss[:, f0:f1], AF.Exp,
                                         scale=rnk_sb[:, n:n + 1])
                    if cb is not None:
                        nc.gpsimd.tensor_mul(es[:, cb:cb + 128],
                                             es[:, cb:cb + 128].bitcast(f32), btri_sb)
                    if eb is not None:
                        nc.gpsimd.tensor_mul(es[:, eb:eb + 128],
                                             es[:, eb:eb + 128].bitcast(f32), etri_sb)
                    nc.tensor.matmul(ps_sum[:, f0:f1], ones_sb, es[:, f0:f1],
                                     start=(idx == 0), stop=(idx == last))
                    nc.tensor.matmul(ps_out[:, f0:f1], vn_sb[:, n, :], es[:, f0:f1],
                                     start=(idx == 0), stop=(idx == last))
                rsum = row_p.tile([1, TS], f32, tag="rsum")
                nc.vector.reciprocal(rsum, ps_sum)
                sbc = bc_p.tile([128, TS], f32, tag="bc")
                nc.gpsimd.partition_broadcast(sbc, rsum)
                yt = yt_p.tile([128, TS], f32r, tag=f"yt{h}")
                nc.vector.tensor_mul(yt, ps_out, sbc)
                yts.append(yt)

            # ---- c_proj partial: outT[co, t] = sum_h wpT[h].T @ yT[h] ----
            if m4 == 0:
                for h in range(HPC):
                    nc.sync.dma_start(out=wp_sb[:, h, :],
                                      in_=wp[h * 128:(h + 1) * 128, :])
            for co in range(NTT):
                ps_p = ps_s.tile([128, TS], f32, tag="s")
                for h in range(HPC):
                    nc.tensor.matmul(ps_p, wp_sb[:, h, co * 128:(co + 1) * 128],
                                     yts[h], start=(h == 0), stop=(h == HPC - 1))
                ot = ot_p.tile([128, TS], f32, tag="ot")
                nc.vector.tensor_copy(ot, ps_p)
                nc.sync.dma_start(out=outT[co * 128:(co + 1) * 128, t0:t0 + TS],
                                  in_=ot)

    # Restrict the activation-table picker to the one set containing every
    # ACT function we use (exp, ln, square, copy, identity): without this the
    # greedy picker alternates exp_and_others <-> natural_log, inserting a
    # ~1.3us table load per switch. Set ids are positions in act_info.json's
    # list, so unwanted sets are emptied rather than removed.
    import concourse.hw_specs as hw_specs
    import concourse.bacc as bacc_mod

    orig = hw_specs.get_activation_tables

    def only_combined(arch):
        t = orig(arch)
        return {k: (v if k == "natural_log_exp_and_others" else set())
                for k, v in t.items()}

    hw_specs.get_activation_tables = only_combined
    bacc_mod.get_activation_tables = only_combined
    try:
        nc.compile()
    finally:
        hw_specs.get_activation_tables = orig
        bacc_mod.get_activation_tables = orig
    return nc


def _prep_inputs(x, ve, cos, sin, Wq, Wk, Wv, Wproj, Wgate, W):
    cosT = np.ascontiguousarray(cos[0, :, 0, :].T)  # (64, T)
    sinT = np.ascontiguousarray(sin[0, :, 0, :].T)
    cc = np.concatenate([cosT, cosT], axis=0).astype(np.float32)
    ss = np.concatenate([sinT, -sinT], axis=0).astype(np.float32)
    p = np.arange(128)[:, None]
    f = np.arange(128)[None, :]
    btri = (p <= f).astype(np.float32)
    etri = (f <= p + (W % 128)).astype(np.float32)

    in_maps = []
    for core in range(8):
        b, g = core // NKV, core % NKV
        hs = slice(g * HPC * HD, (g + 1) * HPC * HD)
        ks = slice(g * HD, (g + 1) * HD)
        in_maps.append({
            "xT": np.ascontiguousarray(x[b].T),
            "wqT": np.ascontiguousarray(Wq[hs, :].T),
            "wkT": np.ascontiguousarray(Wk[ks, :].T),
            "wvT": np.ascontiguousarray(Wv[ks, :].T),
            "wpT": np.ascontiguousarray(Wproj[:, hs].T),
            "wg": np.ascontiguousarray(Wgate[g][:, None]),
            "cc": cc,
            "ss": ss,
            "ve": np.ascontiguousarray(ve[b][:, ks]),
            "btri": btri,
            "etri": etri,
        })
    return in_maps


def _run(inputs, trace=False):
    from concourse.bass_utils import run_bass_kernel_spmd

    x = np.asarray(inputs["x"], dtype=np.float32)
    ve = np.asarray(inputs["ve"], dtype=np.float32)
    cos = np.asarray(inputs["cos"], dtype=np.float32)
    sin = np.asarray(inputs["sin"], dtype=np.float32)
    Wq = np.asarray(inputs["Wq"], dtype=np.float32)
    Wk = np.asarray(inputs["Wk"], dtype=np.float32)
    Wv = np.asarray(inputs["Wv"], dtype=np.float32)
    Wproj = np.asarray(inputs["Wproj"], dtype=np.float32)
    Wgate = np.asarray(inputs["Wgate"], dtype=np.float32)
    W = int(inputs["window_size"])

    if W not in _compiled:
        _compiled[W] = _build(W)
    nc = _compiled[W]

    in_maps = _prep_inputs(x, ve, cos, sin, Wq, Wk, Wv, Wproj, Wgate, W)
    res = run_bass_kernel_spmd(nc, in_maps, core_ids=list(range(8)), trace=trace)

    out = np.zeros((B, T, C), dtype=np.float32)
    for core in range(8):
        b = core // NKV
        out[b] += res.results[core]["outT"].T
    return out, res


def kernel(**inputs):
    out, _ = _run(inputs, trace=False)
    return out



# revision 30
# speedup vs baseline: 1.2161x; 1.2161x over previous
"""Sliding-window causal self-attention (GQA + RoPE + QK-RMSNorm + ve-gate) on
8 Trainium2 NeuronCores.

Sharding: core c handles (batch b = c // 4, kv-head g = c % 4): data parallel
over batch x tensor parallel over the 4 KV head groups (4 query heads per
core). Each core computes its partial c_proj output; the all-reduce over the 4
head shards is a host-side sum.

Device design (per core), v2:
  - x, Wq/Wk/Wv/Wg/Wproj are fed in bf16 (halves DMA + SBUF; matmul rate is
    unchanged and PSUM accumulation stays fp32). The attention inner product
    path (kT/q4/es/vn) stays float32r.
  - q for all 4 heads lives in ONE SBUF tile q4 [128, 4, TS]; scores / ones /
    PV matmuls process all 4 heads per k-tile with [128, 4*128] outputs, so
    every fp32r matmul has a moving free-size of 512 (full PE rate) and the
    instruction count is 1/4 of the per-head variant.
  - scores are computed TRANSPOSED (S^T: tk x tq) so softmax denominators come
    from a ones-matmul and P@V needs no transposes.
  - softmax skips max-subtraction: QK RMS-norm bounds |scores| so exp() cannot
    overflow in fp32. Sliding-window masking multiplies the two triangular
    boundary tiles by 0/1 masks (btri4/etri4, pre-replicated over heads).
  - k's rms-norm scale rides the per-partition `scale` operand of the Exp
    activation; q's rides the PSUM-evacuation multiply.
  - c_proj runs at q-subtile granularity (moving operand yt4 is bf16, so
    128-column matmuls still run at 1 cycle/row) and is emitted interleaved
    into the NEXT attention subtile as PE filler work.
"""

import sys

sys.path.insert(0, "/opt/trn_rl_repo")

import numpy as np

B, T, C = 2, 2048, 2048
NH, NKV, HD = 16, 4, 128
GATE_CH = 12
HPC = NH // NKV          # q heads per core
TS = 512                 # token-slice width
NSL = T // TS            # 4 slices
NCK = C // 128           # 16 contraction chunks
TPS = TS // 128          # 4 token tiles per slice
NTT = T // 128           # 16 token tiles
EPS = 1e-6

A_Q = 1.2 / np.sqrt(float(HD))   # rms-norm scale folded into q (incl 1/sqrt(HD))
A_K = 1.2                        # rms-norm scale folded into exp() scale arg
S_Q = float(1.0 / (HD * A_Q * A_Q))
B_Q = float(EPS / (A_Q * A_Q))
S_K = float(1.0 / (HD * A_K * A_K))
B_K = float(EPS / (A_K * A_K))
LN3I = float(np.log(1.0 / 3.0))

_compiled = {}


def _build(W):
    import concourse.bass as bass
    import concourse.tile as tile
    from concourse import bacc, bass_isa, mybir
    from concourse.masks import make_identity
    from contextlib import ExitStack

    f32 = mybir.dt.float32
    f32r = mybir.dt.float32r
    bf16 = mybir.dt.bfloat16
    AF = mybir.ActivationFunctionType
    OP = mybir.AluOpType

    NW = W // 128            # window in 128-tiles (8)
    assert W % 128 == 0

    nc = bacc.Bacc(None, target_bir_lowering=False)

    xT = nc.dram_tensor("xT", [C, T], bf16, kind="ExternalInput")
    wq = nc.dram_tensor("wqT", [C, HPC * HD], bf16, kind="ExternalInput")
    wk = nc.dram_tensor("wkT", [C, HD], bf16, kind="ExternalInput")
    wv = nc.dram_tensor("wvT", [C, HD], bf16, kind="ExternalInput")
    wp = nc.dram_tensor("wpT", [HPC * HD, C], bf16, kind="ExternalInput")
    wgd = nc.dram_tensor("wg", [GATE_CH, 1], bf16, kind="ExternalInput")
    ccd = nc.dram_tensor("cc", [HD, T], f32, kind="ExternalInput")
    ssd = nc.dram_tensor("ss", [HD, T], f32, kind="ExternalInput")
    ved = nc.dram_tensor("ve", [T, HD], f32, kind="ExternalInput")
    btrid = nc.dram_tensor("btri4", [128, HPC * 128], f32, kind="ExternalInput")
    etrid = nc.dram_tensor("etri4", [128, HPC * 128], f32, kind="ExternalInput")
    permd = nc.dram_tensor("perm", [128, 128], f32r, kind="ExternalInput")
    outT = nc.dram_tensor("outT", [C, T], f32, kind="ExternalOutput")

    with tile.TileContext(nc) as tc, ExitStack() as ctx:
        res = ctx.enter_context(tc.tile_pool(name="res", bufs=1))
        xs_p = ctx.enter_context(tc.tile_pool(name="xs", bufs=2))
        tab_p = ctx.enter_context(tc.tile_pool(name="tab", bufs=2))
        work_p = ctx.enter_context(tc.tile_pool(name="work", bufs=2))
        sq_p = ctx.enter_context(tc.tile_pool(name="sq", bufs=2))
        bc_p = ctx.enter_context(tc.tile_pool(name="bc", bufs=2))
        es_p = ctx.enter_context(tc.tile_pool(name="es", bufs=4))
        q4_p = ctx.enter_context(tc.tile_pool(name="q4", bufs=2))
        yt_p = ctx.enter_context(tc.tile_pool(name="yt", bufs=2))
        ot_p = ctx.enter_context(tc.tile_pool(name="ot", bufs=2))
        row_p = ctx.enter_context(tc.tile_pool(name="rows", bufs=2))

        ps_qkv = ctx.enter_context(tc.tile_pool(name="ps_qkv", bufs=2, space="PSUM"))
        ps_s = ctx.enter_context(tc.tile_pool(name="ps_s", bufs=2, space="PSUM"))
        ps_y = ctx.enter_context(tc.tile_pool(name="ps_y", bufs=1, space="PSUM"))
        ps_p = ctx.enter_context(tc.tile_pool(name="ps_p", bufs=2, space="PSUM"))
        ps_row = ctx.enter_context(tc.tile_pool(name="ps_row", bufs=1, space="PSUM"))
        dram_p = ctx.enter_context(tc.tile_pool(name="dram", bufs=2, space="DRAM"))

        # ---- resident tensors ----
        wg_sb = res.tile([GATE_CH, 1], bf16)
        nc.sync.dma_start(out=wg_sb, in_=wgd[:, :])
        wk_sb = res.tile([128, NCK, HD], bf16)
        xs0 = xs_p.tile([128, NCK, TS], bf16, tag="xs")
        # first parts small so the first k-proj matmuls can start early
        for c0, c1 in ((0, 1), (1, 6), (6, 11), (11, 16)):
            nc.sync.dma_start(
                out=wk_sb[:, c0:c1, :],
                in_=wk[128 * c0:128 * c1, :].rearrange(
                    "(c p) h -> p c h", p=128),
            )
            nc.sync.dma_start(
                out=xs0[:, c0:c1, :],
                in_=xT[128 * c0:128 * c1, 0:TS].rearrange(
                    "(c p) t -> p c t", p=128),
            )
        wq_sb = res.tile([128, NCK, HPC * HD], bf16)
        for p4 in range(4):
            nc.sync.dma_start(
                out=wq_sb[:, 4 * p4:4 * p4 + 4, :],
                in_=wq[512 * p4:512 * (p4 + 1), :].rearrange(
                    "(c p) h -> p c h", p=128),
            )
        wv_sb = res.tile([128, NCK, HD], bf16)
        nc.sync.dma_start(
            out=wv_sb, in_=wv[:, :].rearrange("(c p) h -> p c h", p=128))
        cc0 = tab_p.tile([128, TS], f32, tag="cc")
        nc.sync.dma_start(out=cc0, in_=ccd[:, 0:TS])
        ss0 = tab_p.tile([128, TS], f32, tag="ss")
        nc.sync.dma_start(out=ss0, in_=ssd[:, 0:TS])
        ve0 = tab_p.tile([128, TPS, HD], f32, tag="ve")
        nc.sync.dma_start(
            out=ve0, in_=ved[0:TS, :].rearrange("(tt p) h -> p tt h", p=128))
        btri_sb = res.tile([128, HPC * 128], f32)
        nc.sync.dma_start(out=btri_sb, in_=btrid[:, :])
        etri_sb = res.tile([128, HPC * 128], f32)
        nc.sync.dma_start(out=etri_sb, in_=etrid[:, :])
        perm_sb = res.tile([128, 128], f32r)
        nc.sync.dma_start(out=perm_sb, in_=permd[:, :])
        wp_sb = res.tile([128, HPC, C], bf16)
        for h in range(HPC):
            nc.sync.dma_start(out=wp_sb[:, h, :], in_=wp[h * 128:(h + 1) * 128, :])

        ident = res.tile([128, 128], f32)
        make_identity(nc, ident)
        ones_f = res.tile([128, 1], f32)
        nc.vector.memset(ones_f, 1.0)
        ones_sb = ones_f.bitcast(f32r)
        bq_sb = res.tile([128, 1], f32)
        nc.vector.memset(bq_sb, B_Q)
        bk_sb = res.tile([128, 1], f32)
        nc.vector.memset(bk_sb, B_K)
        bg_sb = res.tile([1, 1], f32)
        nc.vector.memset(bg_sb, LN3I)

        kT_sb = res.tile([128, T], f32r)        # rotated+normalized k, HD on partitions
        vn_sb = res.tile([128, NTT, HD], f32r)  # v natural, token tiles on partitions

        tabs = {0: (cc0, ss0, ve0)}
        xss = {0: xs0}
        yt_tiles = {}

        def rope_half(dst_f32r, cc_sl, ss_sl, tag, scale_bc=None):
            """dst [128, TS] f32r pre-rotation. In-place RoPE; the half-swap
            runs as a PE permutation matmul (no DMA latency). The final write
            goes through the f32r view (required by consuming f32r matmuls)."""
            dst = dst_f32r.bitcast(f32)
            psw = ps_s.tile([128, HPC * 128], f32, tag="s")
            nc.tensor.matmul(psw[:, 0:TS], perm_sb, dst_f32r,
                             start=True, stop=True)
            tmp = work_p.tile([128, TS], f32, tag=tag + "t")
            nc.vector.tensor_mul(tmp, psw[:, 0:TS], ss_sl)
            nc.vector.tensor_mul(dst_f32r, dst, cc_sl)
            nc.vector.tensor_add(dst_f32r, dst, tmp)
            if scale_bc is not None:
                nc.vector.tensor_mul(dst_f32r, dst, scale_bc)

        def emit_cproj(m, j):
            """c_proj for q-subtile j of slice m (yt_tiles[(m, j)] ready)."""
            yt4 = yt_tiles.pop((m, j))
            t0 = m * TS
            ot = ot_p.tile([128, NTT, 128], f32, tag="ot")
            for gco in range(4):
                pp = ps_p.tile([128, 4 * 128], f32, tag="pp")
                for ci in range(4):
                    co = 4 * gco + ci
                    for h in range(HPC):
                        nc.tensor.matmul(
                            pp[:, ci * 128:(ci + 1) * 128],
                            wp_sb[:, h, co * 128:(co + 1) * 128],
                            yt4[:, h, :],
                            start=(h == 0), stop=(h == HPC - 1))
                if gco % 2 == 0:
                    nc.scalar.activation(ot[:, 4 * gco:4 * gco + 4, :], pp, AF.Copy)
                else:
                    nc.vector.tensor_copy(ot[:, 4 * gco:4 * gco + 4, :], pp)
            nc.sync.dma_start(
                out=outT[:, t0 + j * 128:t0 + (j + 1) * 128].rearrange(
                    "(co p) t -> p co t", p=128),
                in_=ot)

        for m in range(NSL):
            t0 = m * TS
            xs = xss.pop(m)
            cc_sl, ss_sl, ve_sl = tabs.pop(m)

            # ---- k projection + rms cols + rope ----
            ps_k = ps_qkv.tile([128, TS], f32, tag="qkv")
            for c in range(NCK):
                nc.tensor.matmul(ps_k, wk_sb[:, c, :], xs[:, c, :],
                                 start=(c == 0), stop=(c == NCK - 1))
            sq_k = sq_p.tile([128, TS], f32, tag="sq")
            nc.scalar.activation(sq_k, ps_k, AF.Square)
            # rsqrt(mean(k^2)) broadcast across partitions; k is pre-normalized
            # (scale folded into the rope's final multiply), so the exp() scale
            # becomes the constant 1.0.
            rbk = bc_p.tile([128, TS], f32, tag="bc")
            nc.gpsimd.partition_all_reduce(rbk, sq_k, channels=128,
                                           reduce_op=bass_isa.ReduceOp.add)
            nc.scalar.activation(rbk, rbk, AF.Ln, bias=bk_sb, scale=S_K)
            nc.scalar.activation(rbk, rbk, AF.Exp, scale=-0.5)
            k_sl = kT_sb[:, t0:t0 + TS]
            nc.vector.tensor_copy(k_sl, ps_k)
            rope_half(k_sl, cc_sl, ss_sl, "ksw", scale_bc=rbk)

            # ---- gate row: 3*sigmoid(x[:, :12] @ wg) ----
            ps_g = ps_row.tile([1, TS], f32, tag="rows")
            nc.tensor.matmul(ps_g, wg_sb, xs[0:GATE_CH, 0, :], start=True, stop=True)
            g_row = row_p.tile([1, TS], f32, tag="grow")
            # e^(-x)/3, then +1/3, then reciprocal => 3*sigmoid(x)
            nc.scalar.activation(g_row, ps_g, AF.Exp, scale=-1.0, bias=bg_sb)
            nc.vector.tensor_scalar(out=g_row, in0=g_row, scalar1=1.0 / 3.0,
                                    scalar2=None, op0=OP.add)
            nc.vector.reciprocal(g_row, g_row)
            g_dr = dram_p.tile([TS], f32, tag="gdr")
            nc.sync.dma_start(out=g_dr, in_=g_row)
            gate_c = row_p.tile([128, TPS], f32, tag="gate")
            nc.sync.dma_start(
                out=gate_c,
                in_=bass.AP(tensor=g_dr.tensor, offset=g_dr.offset,
                            ap=[[1, 128], [128, TPS]]),
            )

            # ---- q projections (4 heads) + rms-norm + rope ----
            # The rms-norm scale is applied as the LAST rope step so the swap
            # DMAs + rotation can proceed in parallel with the row chain.
            q4 = q4_p.tile([128, HPC, TS], f32r, tag="q4")
            for h in range(HPC):
                ps_q = ps_qkv.tile([128, TS], f32, tag="qkv")
                for c in range(NCK):
                    nc.tensor.matmul(ps_q, wq_sb[:, c, h * HD:(h + 1) * HD],
                                     xs[:, c, :],
                                     start=(c == 0), stop=(c == NCK - 1))
                nc.vector.tensor_copy(q4[:, h, :], ps_q)
                sq_q = sq_p.tile([128, TS], f32, tag="sq")
                nc.scalar.activation(sq_q, ps_q, AF.Square)
                rbc = bc_p.tile([128, TS], f32, tag="bc")
                nc.gpsimd.partition_all_reduce(rbc, sq_q,
                                               channels=128,
                                               reduce_op=bass_isa.ReduceOp.add)
                nc.scalar.activation(rbc, rbc, AF.Ln, bias=bq_sb, scale=S_Q)
                nc.scalar.activation(rbc, rbc, AF.Exp, scale=-0.5)
                rope_half(q4[:, h, :], cc_sl, ss_sl, "qsw", scale_bc=rbc)
            if m > 0:
                emit_cproj(m - 1, TPS - 1)

            # ---- prefetch next slice (after rope swaps are queued) ----
            if m + 1 < NSL:
                t1 = (m + 1) * TS
                xs_n = xs_p.tile([128, NCK, TS], bf16, tag="xs")
                for p4 in range(4):
                    nc.sync.dma_start(
                        out=xs_n[:, 4 * p4:4 * p4 + 4, :],
                        in_=xT[512 * p4:512 * (p4 + 1), t1:t1 + TS].rearrange(
                            "(c p) t -> p c t", p=128),
                    )
                cc_n = tab_p.tile([128, TS], f32, tag="cc")
                nc.sync.dma_start(out=cc_n, in_=ccd[:, t1:t1 + TS])
                ss_n = tab_p.tile([128, TS], f32, tag="ss")
                nc.sync.dma_start(out=ss_n, in_=ssd[:, t1:t1 + TS])
                ve_n = tab_p.tile([128, TPS, HD], f32, tag="ve")
                nc.sync.dma_start(
                    out=ve_n,
                    in_=ved[t1:t1 + TS, :].rearrange("(tt p) h -> p tt h", p=128))
                xss[m + 1] = xs_n
                tabs[m + 1] = (cc_n, ss_n, ve_n)

            # ---- v projection + transpose to natural + gate-add ----
            ps_v = ps_qkv.tile([128, TS], f32, tag="qkv")
            for c in range(NCK):
                nc.tensor.matmul(ps_v, wv_sb[:, c, :], xs[:, c, :],
                                 start=(c == 0), stop=(c == NCK - 1))
            vT_s = work_p.tile([128, TS], f32, tag="vt")
            nc.scalar.activation(vT_s, ps_v, AF.Copy)
            ps_t = ps_qkv.tile([128, TS], f32, tag="qkv")
            for tt in range(TPS):
                nc.tensor.transpose(ps_t[:, tt * 128:(tt + 1) * 128],
                                    vT_s[:, tt * 128:(tt + 1) * 128], ident)
            for tt in range(TPS):
                nc.vector.scalar_tensor_tensor(
                    out=vn_sb[:, m * TPS + tt, :],
                    in0=ve_sl[:, tt, :], scalar=gate_c[:, tt:tt + 1],
                    in1=ps_t[:, tt * 128:(tt + 1) * 128],
                    op0=OP.mult, op1=OP.add)

            # ---- attention subtiles (+ interleaved c_proj of previous one) ----
            for j in range(TPS):
                t = m * TPS + j
                nlo = max(0, t - NW)
                # Order: one full tile opens the PSUM groups (short dep chain),
                # the masked boundary tiles (diag/edge) go next so their longer
                # exp->mask chains overlap the remaining full tiles' work.
                fulls = [n for n in range(nlo, t + 1)
                         if n != t and n != t - NW]
                ns = []
                if fulls:
                    ns.append(fulls[0])
                ns.append(t)                      # diag (btri)
                if t - NW >= 0:
                    ns.append(t - NW)             # edge (etri)
                ns.extend(fulls[1:])
                last = len(ns) - 1
                psy = ps_y.tile([128, HPC * 128], f32, tag="py")
                ps_sum = ps_row.tile([1, TS], f32, tag="rows")
                for idx, n in enumerate(ns):
                    pss = ps_s.tile([128, HPC * 128], f32, tag="s")
                    nc.tensor.matmul(pss, kT_sb[:, n * 128:(n + 1) * 128],
                                     q4[:, :, j * 128:(j + 1) * 128],
                                     start=True, stop=True)
                    es = es_p.tile([128, HPC * 128], f32r, tag="es")
                    nc.scalar.activation(es, pss, AF.Exp)
                    if n == t:
                        nc.gpsimd.tensor_mul(es, es.bitcast(f32), btri_sb)
                    if n == t - NW:
                        nc.gpsimd.tensor_mul(es, es.bitcast(f32), etri_sb)
                    nc.tensor.matmul(ps_sum, ones_sb, es,
                                     start=(idx == 0), stop=(idx == last))
                    nc.tensor.matmul(psy, vn_sb[:, n, :], es,
                                     start=(idx == 0), stop=(idx == last))
                rsum = row_p.tile([1, TS], f32, tag="rsum")
                nc.vector.reciprocal(rsum, ps_sum)
                sbc = bc_p.tile([128, TS], f32, tag="sbc")
                nc.gpsimd.partition_broadcast(sbc, rsum)
                yt4 = yt_p.tile([128, HPC, 128], bf16, tag="yt")
                nc.vector.tensor_mul(
                    yt4.rearrange("p h t -> p (h t)"), psy, sbc)
                yt_tiles[(m, j)] = yt4
                if j > 0:
                    emit_cproj(m, j - 1)

        emit_cproj(NSL - 1, TPS - 1)

    # Restrict the activation-table picker to the one set containing every
    # ACT function we use (exp, ln, square, copy): without this the greedy
    # picker alternates tables, inserting a ~1.3us table load per switch.
    import concourse.hw_specs as hw_specs
    import concourse.bacc as bacc_mod

    orig = hw_specs.get_activation_tables

    def only_combined(arch):
        t = orig(arch)
        return {k: (v if k == "natural_log_exp_and_others" else set())
                for k, v in t.items()}

    hw_specs.get_activation_tables = only_combined
    bacc_mod.get_activation_tables = only_combined
    try:
        nc.compile()
    finally:
        hw_specs.get_activation_tables = orig
        bacc_mod.get_activation_tables = orig
    return nc


def _prep_inputs(x, ve, cos, sin, Wq, Wk, Wv, Wproj, Wgate, W):
    import ml_dtypes

    bf = ml_dtypes.bfloat16
    cosT = np.ascontiguousarray(cos[0, :, 0, :].T)  # (64, T)
    sinT = np.ascontiguousarray(sin[0, :, 0, :].T)
    cc = np.concatenate([cosT, cosT], axis=0).astype(np.float32)
    ss = np.concatenate([sinT, -sinT], axis=0).astype(np.float32)
    p = np.arange(128)[:, None]
    f = np.arange(128)[None, :]
    btri = (p <= f).astype(np.float32)
    etri = (f <= p + (W % 128)).astype(np.float32)
    btri4 = np.ascontiguousarray(np.tile(btri, (1, HPC)))
    etri4 = np.ascontiguousarray(np.tile(etri, (1, HPC)))
    # half-swap permutation: out[p] = in[(p+64) % 128]
    perm = np.zeros((128, 128), dtype=np.float32)
    perm[(np.arange(128) + 64) % 128, np.arange(128)] = 1.0

    in_maps = []
    for core in range(8):
        b, g = core // NKV, core % NKV
        hs = slice(g * HPC * HD, (g + 1) * HPC * HD)
        ks = slice(g * HD, (g + 1) * HD)
        in_maps.append({
            "xT": np.ascontiguousarray(x[b].T).astype(bf),
            "wqT": np.ascontiguousarray(Wq[hs, :].T).astype(bf),
            "wkT": np.ascontiguousarray(Wk[ks, :].T).astype(bf),
            "wvT": np.ascontiguousarray(Wv[ks, :].T).astype(bf),
            "wpT": np.ascontiguousarray(Wproj[:, hs].T).astype(bf),
            "wg": np.ascontiguousarray(Wgate[g][:, None]).astype(bf),
            "cc": cc,
            "ss": ss,
            "ve": np.ascontiguousarray(ve[b][:, ks]),
            "btri4": btri4,
            "etri4": etri4,
            "perm": perm,
        })
    return in_maps


def _run(inputs, trace=False):
    from concourse.bass_utils import run_bass_kernel_spmd

    x = np.asarray(inputs["x"], dtype=np.float32)
    ve = np.asarray(inputs["ve"], dtype=np.float32)
    cos = np.asarray(inputs["cos"], dtype=np.float32)
    sin = np.asarray(inputs["sin"], dtype=np.float32)
    Wq = np.asarray(inputs["Wq"], dtype=np.float32)
    Wk = np.asarray(inputs["Wk"], dtype=np.float32)
    Wv = np.asarray(inputs["Wv"], dtype=np.float32)
    Wproj = np.asarray(inputs["Wproj"], dtype=np.float32)
    Wgate = np.asarray(inputs["Wgate"], dtype=np.float32)
    W = int(inputs["window_size"])

    if W not in _compiled:
        _compiled[W] = _build(W)
    nc = _compiled[W]

    in_maps = _prep_inputs(x, ve, cos, sin, Wq, Wk, Wv, Wproj, Wgate, W)
    res = run_bass_kernel_spmd(nc, in_maps, core_ids=list(range(8)), trace=trace)

    out = np.zeros((B, T, C), dtype=np.float32)
    for core in range(8):
        b = core // NKV
        out[b] += res.results[core]["outT"].T
    return out, res


def kernel(**inputs):
    out, _ = _run(inputs, trace=False)
    return out


# revision 42
# speedup vs baseline: 1.2709x; 1.0451x over previous
"""Sliding-window causal self-attention (GQA + RoPE + QK-RMSNorm + ve-gate) on
8 Trainium2 NeuronCores.

Sharding: core c handles (batch b = c // 4, kv-head g = c % 4): data parallel
over batch x tensor parallel over the 4 KV head groups (4 query heads per
core). Each core computes its partial c_proj output; the all-reduce over the 4
head shards is a host-side sum.

Device design (per core), v2:
  - x, Wq/Wk/Wv/Wg/Wproj are fed in bf16 (halves DMA + SBUF; matmul rate is
    unchanged and PSUM accumulation stays fp32). The attention inner product
    path (kT/q4/es/vn) stays float32r.
  - q for all 4 heads lives in ONE SBUF tile q4 [128, 4, TS]; scores / ones /
    PV matmuls process all 4 heads per k-tile with [128, 4*128] outputs, so
    every fp32r matmul has a moving free-size of 512 (full PE rate) and the
    instruction count is 1/4 of the per-head variant.
  - scores are computed TRANSPOSED (S^T: tk x tq) so softmax denominators come
    from a ones-matmul and P@V needs no transposes.
  - softmax skips max-subtraction: QK RMS-norm bounds |scores| so exp() cannot
    overflow in fp32. Sliding-window masking multiplies the two triangular
    boundary tiles by 0/1 masks (btri4/etri4, pre-replicated over heads).
  - k's rms-norm scale rides the per-partition `scale` operand of the Exp
    activation; q's rides the PSUM-evacuation multiply.
  - c_proj runs at q-subtile granularity (moving operand yt4 is bf16, so
    128-column matmuls still run at 1 cycle/row) and is emitted interleaved
    into the NEXT attention subtile as PE filler work.
"""

import sys

sys.path.insert(0, "/opt/trn_rl_repo")

import numpy as np

B, T, C = 2, 2048, 2048
NH, NKV, HD = 16, 4, 128
GATE_CH = 12
HPC = NH // NKV          # q heads per core
TS = 512                 # token-slice width
NSL = T // TS            # 4 slices
NCK = C // 128           # 16 contraction chunks
TPS = TS // 128          # 4 token tiles per slice
NTT = T // 128           # 16 token tiles
EPS = 1e-6

A_Q = 1.2 / np.sqrt(float(HD))   # rms-norm scale folded into q (incl 1/sqrt(HD))
A_K = 1.2                        # rms-norm scale folded into exp() scale arg
S_Q = float(1.0 / (HD * A_Q * A_Q))
B_Q = float(EPS / (A_Q * A_Q))
S_K = float(1.0 / (HD * A_K * A_K))
B_K = float(EPS / (A_K * A_K))
LN3I = float(np.log(1.0 / 3.0))

_compiled = {}


def _build(W):
    import concourse.bass as bass
    import concourse.tile as tile
    from concourse import bacc, bass_isa, mybir
    from concourse.masks import make_identity
    from contextlib import ExitStack

    f32 = mybir.dt.float32
    f32r = mybir.dt.float32r
    bf16 = mybir.dt.bfloat16
    AF = mybir.ActivationFunctionType
    OP = mybir.AluOpType

    NW = W // 128            # window in 128-tiles (8)
    assert W % 128 == 0

    nc = bacc.Bacc(None, target_bir_lowering=False)

    xT = nc.dram_tensor("xT", [C, T], bf16, kind="ExternalInput")
    wq = nc.dram_tensor("wqT", [C, HPC * HD], bf16, kind="ExternalInput")
    wk = nc.dram_tensor("wkT", [C, HD], bf16, kind="ExternalInput")
    wv = nc.dram_tensor("wvT", [C, HD], bf16, kind="ExternalInput")
    wp = nc.dram_tensor("wpT", [HPC * HD, C], bf16, kind="ExternalInput")
    wgd = nc.dram_tensor("wg", [GATE_CH, 1], bf16, kind="ExternalInput")
    ccd = nc.dram_tensor("cc", [HD, T], f32, kind="ExternalInput")
    ssd = nc.dram_tensor("ss", [HD, T], f32, kind="ExternalInput")
    ved = nc.dram_tensor("ve", [T, HD], f32, kind="ExternalInput")
    btrid = nc.dram_tensor("btri4", [128, HPC * 128], f32, kind="ExternalInput")
    etrid = nc.dram_tensor("etri4", [128, HPC * 128], f32, kind="ExternalInput")
    permd = nc.dram_tensor("perm", [128, 128], f32r, kind="ExternalInput")
    outT = nc.dram_tensor("outT", [C, T], f32, kind="ExternalOutput")

    with tile.TileContext(nc) as tc, ExitStack() as ctx:
        res = ctx.enter_context(tc.tile_pool(name="res", bufs=1))
        xs_p = ctx.enter_context(tc.tile_pool(name="xs", bufs=2))
        tab_p = ctx.enter_context(tc.tile_pool(name="tab", bufs=2))
        work_p = ctx.enter_context(tc.tile_pool(name="work", bufs=2))
        sq_p = ctx.enter_context(tc.tile_pool(name="sq", bufs=2))
        bc_p = ctx.enter_context(tc.tile_pool(name="bc", bufs=2))
        es_p = ctx.enter_context(tc.tile_pool(name="es", bufs=4))
        q4_p = ctx.enter_context(tc.tile_pool(name="q4", bufs=2))
        yt_p = ctx.enter_context(tc.tile_pool(name="yt", bufs=2))
        ot_p = ctx.enter_context(tc.tile_pool(name="ot", bufs=2))
        row_p = ctx.enter_context(tc.tile_pool(name="rows", bufs=2))

        ps_qkv = ctx.enter_context(tc.tile_pool(name="ps_qkv", bufs=2, space="PSUM"))
        ps_s = ctx.enter_context(tc.tile_pool(name="ps_s", bufs=2, space="PSUM"))
        ps_y = ctx.enter_context(tc.tile_pool(name="ps_y", bufs=1, space="PSUM"))
        ps_p = ctx.enter_context(tc.tile_pool(name="ps_p", bufs=2, space="PSUM"))
        ps_row = ctx.enter_context(tc.tile_pool(name="ps_row", bufs=1, space="PSUM"))
        dram_p = ctx.enter_context(tc.tile_pool(name="dram", bufs=2, space="DRAM"))

        # ---- resident tensors ----
        wg_sb = res.tile([GATE_CH, 1], bf16)
        nc.sync.dma_start(out=wg_sb, in_=wgd[:, :])
        wk_sb = res.tile([128, NCK, HD], bf16)
        xs0 = xs_p.tile([128, NCK, TS], bf16, tag="xs")
        wq_sb = res.tile([128, NCK, HPC * HD], bf16)
        # first parts small so the first k-proj matmuls can start early;
        # wq parts interleave so the q projections can start right after k
        def ldx(c0, c1):
            nc.sync.dma_start(
                out=xs0[:, c0:c1, :],
                in_=xT[128 * c0:128 * c1, 0:TS].rearrange(
                    "(c p) t -> p c t", p=128),
            )
        def ldwq(c0, c1):
            nc.sync.dma_start(
                out=wq_sb[:, c0:c1, :],
                in_=wq[128 * c0:128 * c1, :].rearrange(
                    "(c p) h -> p c h", p=128),
            )
        nc.sync.dma_start(
            out=wk_sb[:, 0:1, :],
            in_=wk[0:128, :].rearrange("(c p) h -> p c h", p=128))
        ldx(0, 1)
        nc.sync.dma_start(
            out=wk_sb[:, 1:16, :],
            in_=wk[128:2048, :].rearrange("(c p) h -> p c h", p=128))
        ldx(1, 6)
        ldwq(0, 3)
        ldx(6, 11)
        ldwq(3, 8)
        ldx(11, 16)
        ldwq(8, 12)
        ldwq(12, 16)
        wv_sb = res.tile([128, NCK, HD], bf16)
        nc.sync.dma_start(
            out=wv_sb, in_=wv[:, :].rearrange("(c p) h -> p c h", p=128))
        cc0 = tab_p.tile([128, TS], f32, tag="cc")
        nc.sync.dma_start(out=cc0, in_=ccd[:, 0:TS])
        ss0 = tab_p.tile([128, TS], f32, tag="ss")
        nc.sync.dma_start(out=ss0, in_=ssd[:, 0:TS])
        ve0 = tab_p.tile([128, TPS, HD], f32, tag="ve")
        nc.sync.dma_start(
            out=ve0, in_=ved[0:TS, :].rearrange("(tt p) h -> p tt h", p=128))
        btri_sb = res.tile([128, HPC * 128], f32)
        nc.sync.dma_start(out=btri_sb, in_=btrid[:, :])
        etri_sb = res.tile([128, HPC * 128], f32)
        nc.sync.dma_start(out=etri_sb, in_=etrid[:, :])
        perm_sb = res.tile([128, 128], f32r)
        nc.sync.dma_start(out=perm_sb, in_=permd[:, :])
        wp_sb = res.tile([128, HPC, C], bf16)
        for h in range(HPC):
            nc.sync.dma_start(out=wp_sb[:, h, :], in_=wp[h * 128:(h + 1) * 128, :])

        ident = res.tile([128, 128], f32)
        make_identity(nc, ident)
        ones_f = res.tile([128, 1], f32)
        nc.vector.memset(ones_f, 1.0)
        ones_sb = ones_f.bitcast(f32r)
        bq_sb = res.tile([128, 1], f32)
        nc.vector.memset(bq_sb, B_Q)
        bk_sb = res.tile([128, 1], f32)
        nc.vector.memset(bk_sb, B_K)
        bg_sb = res.tile([1, 1], f32)
        nc.vector.memset(bg_sb, LN3I)

        kT_sb = res.tile([128, T], f32r)        # rotated+normalized k, HD on partitions
        vn_sb = res.tile([128, NTT, HD], f32r)  # v natural, token tiles on partitions

        tabs = {0: (cc0, ss0, ve0)}
        xss = {0: xs0}
        yt_tiles = {}

        def rope_half(dst_f32r, cc_sl, ss_sl, tag, scale_bc=None):
            """dst [128, TS] f32r pre-rotation. In-place RoPE; the half-swap
            runs as a PE permutation matmul (no DMA latency). The final write
            goes through the f32r view (required by consuming f32r matmuls)."""
            dst = dst_f32r.bitcast(f32)
            psw = ps_s.tile([128, HPC * 128], f32, tag="s")
            nc.tensor.matmul(psw[:, 0:TS], perm_sb, dst_f32r,
                             start=True, stop=True)
            tmp = work_p.tile([128, TS], f32, tag=tag + "t")
            nc.vector.tensor_mul(tmp, psw[:, 0:TS], ss_sl)
            nc.vector.tensor_mul(dst_f32r, dst, cc_sl)
            nc.vector.tensor_add(dst_f32r, dst, tmp)
            if scale_bc is not None:
                nc.vector.tensor_mul(dst_f32r, dst, scale_bc)

        def emit_cproj(m, j):
            """c_proj for q-subtile j of slice m (yt_tiles[(m, j)] ready)."""
            yt4 = yt_tiles.pop((m, j))
            t0 = m * TS
            ot = ot_p.tile([128, NTT, 128], f32, tag="ot")
            for gco in range(4):
                pp = ps_p.tile([128, 4 * 128], f32, tag="pp")
                for ci in range(4):
                    co = 4 * gco + ci
                    for h in range(HPC):
                        nc.tensor.matmul(
                            pp[:, ci * 128:(ci + 1) * 128],
                            wp_sb[:, h, co * 128:(co + 1) * 128],
                            yt4[:, h, :],
                            start=(h == 0), stop=(h == HPC - 1))
                if gco % 2 == 0:
                    nc.scalar.activation(ot[:, 4 * gco:4 * gco + 4, :], pp, AF.Copy)
                else:
                    nc.vector.tensor_copy(ot[:, 4 * gco:4 * gco + 4, :], pp)
            nc.sync.dma_start(
                out=outT[:, t0 + j * 128:t0 + (j + 1) * 128].rearrange(
                    "(co p) t -> p co t", p=128),
                in_=ot)

        for m in range(NSL):
            t0 = m * TS
            xs = xss.pop(m)
            cc_sl, ss_sl, ve_sl = tabs.pop(m)

            # ---- k projection + rms cols + rope ----
            ps_k = ps_qkv.tile([128, TS], f32, tag="qkv")
            for c in range(NCK):
                nc.tensor.matmul(ps_k, wk_sb[:, c, :], xs[:, c, :],
                                 start=(c == 0), stop=(c == NCK - 1))
            sq_k = sq_p.tile([128, TS], f32, tag="sq")
            nc.scalar.activation(sq_k, ps_k, AF.Square)
            # rsqrt(mean(k^2)) broadcast across partitions; k is pre-normalized
            # (scale folded into the rope's final multiply), so the exp() scale
            # becomes the constant 1.0.
            rbk = bc_p.tile([128, TS], f32, tag="bc")
            nc.gpsimd.partition_all_reduce(rbk, sq_k, channels=128,
                                           reduce_op=bass_isa.ReduceOp.add)
            nc.scalar.activation(rbk, rbk, AF.Ln, bias=bk_sb, scale=S_K)
            nc.scalar.activation(rbk, rbk, AF.Exp, scale=-0.5)
            k_sl = kT_sb[:, t0:t0 + TS]
            nc.vector.tensor_copy(k_sl, ps_k)
            rope_half(k_sl, cc_sl, ss_sl, "ksw", scale_bc=rbk)

            # ---- gate row: 3*sigmoid(x[:, :12] @ wg) ----
            ps_g = ps_row.tile([1, TS], f32, tag="rows")
            nc.tensor.matmul(ps_g, wg_sb, xs[0:GATE_CH, 0, :], start=True, stop=True)
            g_row = row_p.tile([1, TS], f32, tag="grow")
            # e^(-x)/3, then +1/3, then reciprocal => 3*sigmoid(x)
            nc.scalar.activation(g_row, ps_g, AF.Exp, scale=-1.0, bias=bg_sb)
            nc.vector.tensor_scalar(out=g_row, in0=g_row, scalar1=1.0 / 3.0,
                                    scalar2=None, op0=OP.add)
            nc.vector.reciprocal(g_row, g_row)
            g_dr = dram_p.tile([TS], f32, tag="gdr")
            nc.sync.dma_start(out=g_dr, in_=g_row)
            gate_c = row_p.tile([128, TPS], f32, tag="gate")
            nc.sync.dma_start(
                out=gate_c,
                in_=bass.AP(tensor=g_dr.tensor, offset=g_dr.offset,
                            ap=[[1, 128], [128, TPS]]),
            )

            # ---- q projections (4 heads) + rms-norm + rope ----
            # The rms-norm scale is applied as the LAST rope step so the swap
            # DMAs + rotation can proceed in parallel with the row chain.
            q4 = q4_p.tile([128, HPC, TS], f32r, tag="q4")
            for h in range(HPC):
                ps_q = ps_qkv.tile([128, TS], f32, tag="qkv")
                for c in range(NCK):
                    nc.tensor.matmul(ps_q, wq_sb[:, c, h * HD:(h + 1) * HD],
                                     xs[:, c, :],
                                     start=(c == 0), stop=(c == NCK - 1))
                nc.vector.tensor_copy(q4[:, h, :], ps_q)
                sq_q = sq_p.tile([128, TS], f32, tag="sq")
                nc.scalar.activation(sq_q, ps_q, AF.Square)
                rbc = bc_p.tile([128, TS], f32, tag="bc")
                nc.gpsimd.partition_all_reduce(rbc, sq_q,
                                               channels=128,
                                               reduce_op=bass_isa.ReduceOp.add)
                nc.scalar.activation(rbc, rbc, AF.Ln, bias=bq_sb, scale=S_Q)
                nc.scalar.activation(rbc, rbc, AF.Exp, scale=-0.5)
                rope_half(q4[:, h, :], cc_sl, ss_sl, "qsw", scale_bc=rbc)
            if m > 0:
                emit_cproj(m - 1, TPS - 1)

            # ---- prefetch next slice (after rope swaps are queued) ----
            if m + 1 < NSL:
                t1 = (m + 1) * TS
                xs_n = xs_p.tile([128, NCK, TS], bf16, tag="xs")
                for p4 in range(4):
                    nc.sync.dma_start(
                        out=xs_n[:, 4 * p4:4 * p4 + 4, :],
                        in_=xT[512 * p4:512 * (p4 + 1), t1:t1 + TS].rearrange(
                            "(c p) t -> p c t", p=128),
                    )
                cc_n = tab_p.tile([128, TS], f32, tag="cc")
                nc.sync.dma_start(out=cc_n, in_=ccd[:, t1:t1 + TS])
                ss_n = tab_p.tile([128, TS], f32, tag="ss")
                nc.sync.dma_start(out=ss_n, in_=ssd[:, t1:t1 + TS])
                ve_n = tab_p.tile([128, TPS, HD], f32, tag="ve")
                nc.sync.dma_start(
                    out=ve_n,
                    in_=ved[t1:t1 + TS, :].rearrange("(tt p) h -> p tt h", p=128))
                xss[m + 1] = xs_n
                tabs[m + 1] = (cc_n, ss_n, ve_n)

            # ---- v projection + transpose to natural + gate-add ----
            ps_v = ps_qkv.tile([128, TS], f32, tag="qkv")
            for c in range(NCK):
                nc.tensor.matmul(ps_v, wv_sb[:, c, :], xs[:, c, :],
                                 start=(c == 0), stop=(c == NCK - 1))
            vT_s = work_p.tile([128, TS], f32, tag="vt")
            nc.scalar.activation(vT_s, ps_v, AF.Copy)
            ps_t = ps_qkv.tile([128, TS], f32, tag="qkv")
            for tt in range(TPS):
                nc.tensor.transpose(ps_t[:, tt * 128:(tt + 1) * 128],
                                    vT_s[:, tt * 128:(tt + 1) * 128], ident)
            for tt in range(TPS):
                nc.vector.scalar_tensor_tensor(
                    out=vn_sb[:, m * TPS + tt, :],
                    in0=ve_sl[:, tt, :], scalar=gate_c[:, tt:tt + 1],
                    in1=ps_t[:, tt * 128:(tt + 1) * 128],
                    op0=OP.mult, op1=OP.add)

            # ---- attention subtiles (+ interleaved c_proj of previous one) ----
            for j in range(TPS):
                t = m * TPS + j
                nlo = max(0, t - NW)
                # Order: one full tile opens the PSUM groups (short dep chain),
                # the masked boundary tiles (diag/edge) go next so their longer
                # exp->mask chains overlap the remaining full tiles' work.
                fulls = [n for n in range(nlo, t + 1)
                         if n != t and n != t - NW]
                ns = []
                if fulls:
                    ns.append(fulls[0])
                ns.append(t)                      # diag (btri)
                if t - NW >= 0:
                    ns.append(t - NW)             # edge (etri)
                ns.extend(fulls[1:])
                last = len(ns) - 1
                psy = ps_y.tile([128, HPC * 128], f32, tag="py")
                ps_sum = ps_row.tile([1, TS], f32, tag="rows")
                for idx, n in enumerate(ns):
                    pss = ps_s.tile([128, HPC * 128], f32, tag="s")
                    nc.tensor.matmul(pss, kT_sb[:, n * 128:(n + 1) * 128],
                                     q4[:, :, j * 128:(j + 1) * 128],
                                     start=True, stop=True)
                    es = es_p.tile([128, HPC * 128], f32r, tag="es")
                    nc.scalar.activation(es, pss, AF.Exp)
                    if n == t:
                        nc.vector.tensor_mul(es, es.bitcast(f32), btri_sb)
                    if n == t - NW:
                        nc.vector.tensor_mul(es, es.bitcast(f32), etri_sb)
                    nc.tensor.matmul(ps_sum, ones_sb, es,
                                     start=(idx == 0), stop=(idx == last))
                    nc.tensor.matmul(psy, vn_sb[:, n, :], es,
                                     start=(idx == 0), stop=(idx == last))
                rsum = row_p.tile([1, TS], f32, tag="rsum")
                nc.vector.reciprocal(rsum, ps_sum)
                sbc = bc_p.tile([128, TS], f32, tag="sbc")
                nc.gpsimd.partition_broadcast(sbc, rsum)
                yt4 = yt_p.tile([128, HPC, 128], bf16, tag="yt")
                nc.vector.tensor_mul(
                    yt4.rearrange("p h t -> p (h t)"), psy, sbc)
                yt_tiles[(m, j)] = yt4
                if j > 0:
                    emit_cproj(m, j - 1)

        emit_cproj(NSL - 1, TPS - 1)

    # Restrict the activation-table picker to the one set containing every
    # ACT function we use (exp, ln, square, copy): without this the greedy
    # picker alternates tables, inserting a ~1.3us table load per switch.
    import concourse.hw_specs as hw_specs
    import concourse.bacc as bacc_mod

    orig = hw_specs.get_activation_tables

    def only_combined(arch):
        t = orig(arch)
        return {k: (v if k == "natural_log_exp_and_others" else set())
                for k, v in t.items()}

    hw_specs.get_activation_tables = only_combined
    bacc_mod.get_activation_tables = only_combined
    try:
        nc.compile()
    finally:
        hw_specs.get_activation_tables = orig
        bacc_mod.get_activation_tables = orig
    return nc


def _prep_inputs(x, ve, cos, sin, Wq, Wk, Wv, Wproj, Wgate, W):
    import ml_dtypes

    bf = ml_dtypes.bfloat16
    cosT = np.ascontiguousarray(cos[0, :, 0, :].T)  # (64, T)
    sinT = np.ascontiguousarray(sin[0, :, 0, :].T)
    cc = np.concatenate([cosT, cosT], axis=0).astype(np.float32)
    ss = np.concatenate([sinT, -sinT], axis=0).astype(np.float32)
    p = np.arange(128)[:, None]
    f = np.arange(128)[None, :]
    btri = (p <= f).astype(np.float32)
    etri = (f <= p + (W % 128)).astype(np.float32)
    btri4 = np.ascontiguousarray(np.tile(btri, (1, HPC)))
    etri4 = np.ascontiguousarray(np.tile(etri, (1, HPC)))
    # half-swap permutation: out[p] = in[(p+64) % 128]
    perm = np.zeros((128, 128), dtype=np.float32)
    perm[(np.arange(128) + 64) % 128, np.arange(128)] = 1.0

    in_maps = []
    for core in range(8):
        b, g = core // NKV, core % NKV
        hs = slice(g * HPC * HD, (g + 1) * HPC * HD)
        ks = slice(g * HD, (g + 1) * HD)
        in_maps.append({
            "xT": np.ascontiguousarray(x[b].T).astype(bf),
            "wqT": np.ascontiguousarray(Wq[hs, :].T).astype(bf),
            "wkT": np.ascontiguousarray(Wk[ks, :].T).astype(bf),
            "wvT": np.ascontiguousarray(Wv[ks, :].T).astype(bf),
            "wpT": np.ascontiguousarray(Wproj[:, hs].T).astype(bf),
            "wg": np.ascontiguousarray(Wgate[g][:, None]).astype(bf),
            "cc": cc,
            "ss": ss,
            "ve": np.ascontiguousarray(ve[b][:, ks]),
            "btri4": btri4,
            "etri4": etri4,
            "perm": perm,
        })
    return in_maps


def _run(inputs, trace=False):
    from concourse.bass_utils import run_bass_kernel_spmd

    x = np.asarray(inputs["x"], dtype=np.float32)
    ve = np.asarray(inputs["ve"], dtype=np.float32)
    cos = np.asarray(inputs["cos"], dtype=np.float32)
    sin = np.asarray(inputs["sin"], dtype=np.float32)
    Wq = np.asarray(inputs["Wq"], dtype=np.float32)
    Wk = np.asarray(inputs["Wk"], dtype=np.float32)
    Wv = np.asarray(inputs["Wv"], dtype=np.float32)
    Wproj = np.asarray(inputs["Wproj"], dtype=np.float32)
    Wgate = np.asarray(inputs["Wgate"], dtype=np.float32)
    W = int(inputs["window_size"])

    if W not in _compiled:
        _compiled[W] = _build(W)
    nc = _compiled[W]

    in_maps = _prep_inputs(x, ve, cos, sin, Wq, Wk, Wv, Wproj, Wgate, W)
    res = run_bass_kernel_spmd(nc, in_maps, core_ids=list(range(8)), trace=trace)

    out = np.zeros((B, T, C), dtype=np.float32)
    for core in range(8):
        b = core // NKV
        out[b] += res.results[core]["outT"].T
    return out, res


def kernel(**inputs):
    out, _ = _run(inputs, trace=False)
    return out


# revision 44
# speedup vs baseline: 1.2886x; 1.0139x over previous
"""Sliding-window causal self-attention (GQA + RoPE + QK-RMSNorm + ve-gate) on
8 Trainium2 NeuronCores.

Sharding: core c handles (batch b = c // 4, kv-head g = c % 4): data parallel
over batch x tensor parallel over the 4 KV head groups (4 query heads per
core). Each core computes its partial c_proj output; the all-reduce over the 4
head shards is a host-side sum.

Device design (per core), v2:
  - x, Wq/Wk/Wv/Wg/Wproj are fed in bf16 (halves DMA + SBUF; matmul rate is
    unchanged and PSUM accumulation stays fp32). The attention inner product
    path (kT/q4/es/vn) stays float32r.
  - q for all 4 heads lives in ONE SBUF tile q4 [128, 4, TS]; scores / ones /
    PV matmuls process all 4 heads per k-tile with [128, 4*128] outputs, so
    every fp32r matmul has a moving free-size of 512 (full PE rate) and the
    instruction count is 1/4 of the per-head variant.
  - scores are computed TRANSPOSED (S^T: tk x tq) so softmax denominators come
    from a ones-matmul and P@V needs no transposes.
  - softmax skips max-subtraction: QK RMS-norm bounds |scores| so exp() cannot
    overflow in fp32. Sliding-window masking multiplies the two triangular
    boundary tiles by 0/1 masks (btri4/etri4, pre-replicated over heads).
  - k's rms-norm scale rides the per-partition `scale` operand of the Exp
    activation; q's rides the PSUM-evacuation multiply.
  - c_proj runs at q-subtile granularity (moving operand yt4 is bf16, so
    128-column matmuls still run at 1 cycle/row) and is emitted interleaved
    into the NEXT attention subtile as PE filler work.
"""

import sys

sys.path.insert(0, "/opt/trn_rl_repo")

import numpy as np

B, T, C = 2, 2048, 2048
NH, NKV, HD = 16, 4, 128
GATE_CH = 12
HPC = NH // NKV          # q heads per core
TS = 512                 # token-slice width
NSL = T // TS            # 4 slices
NCK = C // 128           # 16 contraction chunks
TPS = TS // 128          # 4 token tiles per slice
NTT = T // 128           # 16 token tiles
EPS = 1e-6

A_Q = 1.2 / np.sqrt(float(HD))   # rms-norm scale folded into q (incl 1/sqrt(HD))
A_K = 1.2                        # rms-norm scale folded into exp() scale arg
S_Q = float(1.0 / (HD * A_Q * A_Q))
B_Q = float(EPS / (A_Q * A_Q))
S_K = float(1.0 / (HD * A_K * A_K))
B_K = float(EPS / (A_K * A_K))
LN3I = float(np.log(1.0 / 3.0))

_compiled = {}


def _build(W):
    import concourse.bass as bass
    import concourse.tile as tile
    from concourse import bacc, bass_isa, mybir
    from concourse.masks import make_identity
    from contextlib import ExitStack

    f32 = mybir.dt.float32
    f32r = mybir.dt.float32r
    bf16 = mybir.dt.bfloat16
    AF = mybir.ActivationFunctionType
    OP = mybir.AluOpType

    NW = W // 128            # window in 128-tiles (8)
    assert W % 128 == 0

    nc = bacc.Bacc(None, target_bir_lowering=False)

    xT = nc.dram_tensor("xT", [C, T], bf16, kind="ExternalInput")
    wq = nc.dram_tensor("wqT", [C, HPC * HD], bf16, kind="ExternalInput")
    wk = nc.dram_tensor("wkT", [C, HD], bf16, kind="ExternalInput")
    wv = nc.dram_tensor("wvT", [C, HD], bf16, kind="ExternalInput")
    wp = nc.dram_tensor("wpT", [HPC * HD, C], bf16, kind="ExternalInput")
    wgd = nc.dram_tensor("wg", [GATE_CH, 1], bf16, kind="ExternalInput")
    ccd = nc.dram_tensor("cc", [HD, T], f32, kind="ExternalInput")
    ssd = nc.dram_tensor("ss", [HD, T], f32, kind="ExternalInput")
    ved = nc.dram_tensor("ve", [T, HD], f32, kind="ExternalInput")
    btrid = nc.dram_tensor("btri4", [128, HPC * 128], f32, kind="ExternalInput")
    etrid = nc.dram_tensor("etri4", [128, HPC * 128], f32, kind="ExternalInput")
    permd = nc.dram_tensor("perm", [128, 128], f32r, kind="ExternalInput")
    seld = nc.dram_tensor("sel", [128, 3], bf16, kind="ExternalInput")
    outT = nc.dram_tensor("outT", [C, T], f32, kind="ExternalOutput")

    with tile.TileContext(nc) as tc, ExitStack() as ctx:
        res = ctx.enter_context(tc.tile_pool(name="res", bufs=1))
        xs_p = ctx.enter_context(tc.tile_pool(name="xs", bufs=2))
        tab_p = ctx.enter_context(tc.tile_pool(name="tab", bufs=2))
        work_p = ctx.enter_context(tc.tile_pool(name="work", bufs=2))
        sq_p = ctx.enter_context(tc.tile_pool(name="sq", bufs=2))
        bc_p = ctx.enter_context(tc.tile_pool(name="bc", bufs=2))
        es_p = ctx.enter_context(tc.tile_pool(name="es", bufs=3))
        es3_p = ctx.enter_context(tc.tile_pool(name="es3", bufs=2))
        q4_p = ctx.enter_context(tc.tile_pool(name="q4", bufs=2))
        yt_p = ctx.enter_context(tc.tile_pool(name="yt", bufs=2))
        ot_p = ctx.enter_context(tc.tile_pool(name="ot", bufs=2))
        row_p = ctx.enter_context(tc.tile_pool(name="rows", bufs=2))

        ps_qkv = ctx.enter_context(tc.tile_pool(name="ps_qkv", bufs=2, space="PSUM"))
        ps_s = ctx.enter_context(tc.tile_pool(name="ps_s", bufs=2, space="PSUM"))
        ps_y = ctx.enter_context(tc.tile_pool(name="ps_y", bufs=1, space="PSUM"))
        ps_p = ctx.enter_context(tc.tile_pool(name="ps_p", bufs=2, space="PSUM"))
        ps_row = ctx.enter_context(tc.tile_pool(name="ps_row", bufs=1, space="PSUM"))
        dram_p = ctx.enter_context(tc.tile_pool(name="dram", bufs=2, space="DRAM"))

        # ---- resident tensors ----
        wg_sb = res.tile([GATE_CH, 1], bf16)
        nc.sync.dma_start(out=wg_sb, in_=wgd[:, :])
        wk_sb = res.tile([128, NCK, HD], bf16)
        xs0 = xs_p.tile([128, NCK, TS], bf16, tag="xs")
        wq_sb = res.tile([128, NCK, HPC * HD], bf16)
        # first parts small so the first k-proj matmuls can start early;
        # wq parts interleave so the q projections can start right after k
        def ldx(c0, c1):
            nc.sync.dma_start(
                out=xs0[:, c0:c1, :],
                in_=xT[128 * c0:128 * c1, 0:TS].rearrange(
                    "(c p) t -> p c t", p=128),
            )
        def ldwq(c0, c1):
            nc.sync.dma_start(
                out=wq_sb[:, c0:c1, :],
                in_=wq[128 * c0:128 * c1, :].rearrange(
                    "(c p) h -> p c h", p=128),
            )
        nc.sync.dma_start(
            out=wk_sb[:, 0:1, :],
            in_=wk[0:128, :].rearrange("(c p) h -> p c h", p=128))
        ldx(0, 1)
        nc.sync.dma_start(
            out=wk_sb[:, 1:16, :],
            in_=wk[128:2048, :].rearrange("(c p) h -> p c h", p=128))
        ldx(1, 6)
        ldwq(0, 3)
        ldx(6, 11)
        ldwq(3, 8)
        ldx(11, 16)
        ldwq(8, 12)
        ldwq(12, 16)
        wv_sb = res.tile([128, NCK, HD], bf16)
        nc.sync.dma_start(
            out=wv_sb, in_=wv[:, :].rearrange("(c p) h -> p c h", p=128))
        cc0 = tab_p.tile([128, TS], f32, tag="cc")
        nc.sync.dma_start(out=cc0, in_=ccd[:, 0:TS])
        ss0 = tab_p.tile([128, TS], f32, tag="ss")
        nc.sync.dma_start(out=ss0, in_=ssd[:, 0:TS])
        ve0 = tab_p.tile([128, TPS, HD], f32, tag="ve")
        nc.sync.dma_start(
            out=ve0, in_=ved[0:TS, :].rearrange("(tt p) h -> p tt h", p=128))
        btri_sb = res.tile([128, HPC * 128], f32)
        nc.sync.dma_start(out=btri_sb, in_=btrid[:, :])
        etri_sb = res.tile([128, HPC * 128], f32)
        nc.sync.dma_start(out=etri_sb, in_=etrid[:, :])
        perm_sb = res.tile([128, 128], f32r)
        nc.sync.dma_start(out=perm_sb, in_=permd[:, :])
        sel_sb = res.tile([128, 3], bf16)
        nc.sync.dma_start(out=sel_sb, in_=seld[:, :])
        wp_sb = res.tile([128, HPC, C], bf16)
        for h in range(HPC):
            nc.sync.dma_start(out=wp_sb[:, h, :], in_=wp[h * 128:(h + 1) * 128, :])

        ident = res.tile([128, 128], f32)
        make_identity(nc, ident)
        ones_f = res.tile([128, 1], f32)
        nc.vector.memset(ones_f, 1.0)
        ones_sb = ones_f.bitcast(f32r)
        bq_sb = res.tile([128, 1], f32)
        nc.vector.memset(bq_sb, B_Q)
        bk_sb = res.tile([128, 1], f32)
        nc.vector.memset(bk_sb, B_K)
        bg_sb = res.tile([1, 1], f32)
        nc.vector.memset(bg_sb, LN3I)

        rows_ab = []
        for i in range(2):
            r = res.tile([128, 3, TS], bf16, tag=f"rows{i}")
            nc.vector.memset(r, 0.0)
            rows_ab.append(r)
        kT_sb = res.tile([128, T], f32r)        # rotated+normalized k, HD on partitions
        vn_sb = res.tile([128, NTT, HD], f32r)  # v natural, token tiles on partitions

        tabs = {0: (cc0, ss0, ve0)}
        xss = {0: xs0}
        yt_tiles = {}

        def rope_half(dst_f32r, cc_sl, ss_sl, tag, scale_bc=None):
            """dst [128, TS] f32r pre-rotation. In-place RoPE; the half-swap
            runs as a PE permutation matmul (no DMA latency). The final write
            goes through the f32r view (required by consuming f32r matmuls)."""
            dst = dst_f32r.bitcast(f32)
            psw = ps_s.tile([128, HPC * 128], f32, tag="s")
            nc.tensor.matmul(psw[:, 0:TS], perm_sb, dst_f32r,
                             start=True, stop=True)
            tmp = work_p.tile([128, TS], f32, tag=tag + "t")
            nc.vector.tensor_mul(tmp, psw[:, 0:TS], ss_sl)
            nc.vector.tensor_mul(dst_f32r, dst, cc_sl)
            nc.vector.tensor_add(dst_f32r, dst, tmp)
            if scale_bc is not None:
                nc.vector.tensor_mul(dst_f32r, dst, scale_bc)

        def emit_cproj(m, j):
            """c_proj for q-subtile j of slice m (yt_tiles[(m, j)] ready)."""
            yt4 = yt_tiles.pop((m, j))
            t0 = m * TS
            ot = ot_p.tile([128, NTT, 128], f32, tag="ot")
            for gco in range(4):
                pp = ps_p.tile([128, 4 * 128], f32, tag="pp")
                for ci in range(4):
                    co = 4 * gco + ci
                    for h in range(HPC):
                        nc.tensor.matmul(
                            pp[:, ci * 128:(ci + 1) * 128],
                            wp_sb[:, h, co * 128:(co + 1) * 128],
                            yt4[:, h, :],
                            start=(h == 0), stop=(h == HPC - 1))
                if gco % 2 == 0:
                    nc.scalar.activation(ot[:, 4 * gco:4 * gco + 4, :], pp, AF.Copy)
                else:
                    nc.vector.tensor_copy(ot[:, 4 * gco:4 * gco + 4, :], pp)
                nc.sync.dma_start(
                    out=outT[512 * gco:512 * (gco + 1),
                             t0 + j * 128:t0 + (j + 1) * 128].rearrange(
                        "(co p) t -> p co t", p=128),
                    in_=ot[:, 4 * gco:4 * gco + 4, :])

        for m in range(NSL):
            t0 = m * TS
            xs = xss.pop(m)
            cc_sl, ss_sl, ve_sl = tabs.pop(m)

            # ---- k projection + rms cols + rope ----
            ps_k = ps_qkv.tile([128, TS], f32, tag="qkv")
            for c in range(NCK):
                nc.tensor.matmul(ps_k, wk_sb[:, c, :], xs[:, c, :],
                                 start=(c == 0), stop=(c == NCK - 1))
            sq_k = sq_p.tile([128, TS], f32, tag="sq")
            nc.scalar.activation(sq_k, ps_k, AF.Square)
            # rsqrt(mean(k^2)) broadcast across partitions; k is pre-normalized
            # (scale folded into the rope's final multiply), so the exp() scale
            # becomes the constant 1.0.
            rbk = bc_p.tile([128, TS], f32, tag="bc")
            nc.gpsimd.partition_all_reduce(rbk, sq_k, channels=128,
                                           reduce_op=bass_isa.ReduceOp.add)
            nc.scalar.activation(rbk, rbk, AF.Ln, bias=bk_sb, scale=S_K)
            nc.scalar.activation(rbk, rbk, AF.Exp, scale=-0.5)
            k_sl = kT_sb[:, t0:t0 + TS]
            nc.vector.tensor_copy(k_sl, ps_k)
            rope_half(k_sl, cc_sl, ss_sl, "ksw", scale_bc=rbk)

            # ---- gate row: 3*sigmoid(x[:, :12] @ wg) ----
            ps_g = ps_row.tile([1, TS], f32, tag="rows")
            nc.tensor.matmul(ps_g, wg_sb, xs[0:GATE_CH, 0, :], start=True, stop=True)
            g_row = row_p.tile([1, TS], f32, tag="grow")
            # e^(-x)/3, then +1/3, then reciprocal => 3*sigmoid(x)
            nc.scalar.activation(g_row, ps_g, AF.Exp, scale=-1.0, bias=bg_sb)
            nc.vector.tensor_scalar(out=g_row, in0=g_row, scalar1=1.0 / 3.0,
                                    scalar2=None, op0=OP.add)
            nc.vector.reciprocal(g_row, g_row)
            g_dr = dram_p.tile([TS], f32, tag="gdr")
            nc.sync.dma_start(out=g_dr, in_=g_row)
            gate_c = row_p.tile([128, TPS], f32, tag="gate")
            nc.sync.dma_start(
                out=gate_c,
                in_=bass.AP(tensor=g_dr.tensor, offset=g_dr.offset,
                            ap=[[1, 128], [128, TPS]]),
            )

            # ---- q projections (4 heads) + rms-norm + rope ----
            # The rms-norm scale is applied as the LAST rope step so the swap
            # DMAs + rotation can proceed in parallel with the row chain.
            q4 = q4_p.tile([128, HPC, TS], f32r, tag="q4")
            for h in range(HPC):
                ps_q = ps_qkv.tile([128, TS], f32, tag="qkv")
                for c in range(NCK):
                    nc.tensor.matmul(ps_q, wq_sb[:, c, h * HD:(h + 1) * HD],
                                     xs[:, c, :],
                                     start=(c == 0), stop=(c == NCK - 1))
                nc.vector.tensor_copy(q4[:, h, :], ps_q)
                sq_q = sq_p.tile([128, TS], f32, tag="sq")
                nc.scalar.activation(sq_q, ps_q, AF.Square)
                rbc = bc_p.tile([128, TS], f32, tag="bc")
                nc.gpsimd.partition_all_reduce(rbc, sq_q,
                                               channels=128,
                                               reduce_op=bass_isa.ReduceOp.add)
                nc.scalar.activation(rbc, rbc, AF.Ln, bias=bq_sb, scale=S_Q)
                nc.scalar.activation(rbc, rbc, AF.Exp, scale=-0.5)
                rope_half(q4[:, h, :], cc_sl, ss_sl, "qsw", scale_bc=rbc)
            if m > 0:
                emit_cproj(m - 1, TPS - 1)

            # ---- prefetch next slice (after rope swaps are queued) ----
            if m + 1 < NSL:
                t1 = (m + 1) * TS
                xs_n = xs_p.tile([128, NCK, TS], bf16, tag="xs")
                for p4 in range(4):
                    nc.sync.dma_start(
                        out=xs_n[:, 4 * p4:4 * p4 + 4, :],
                        in_=xT[512 * p4:512 * (p4 + 1), t1:t1 + TS].rearrange(
                            "(c p) t -> p c t", p=128),
                    )
                cc_n = tab_p.tile([128, TS], f32, tag="cc")
                nc.sync.dma_start(out=cc_n, in_=ccd[:, t1:t1 + TS])
                ss_n = tab_p.tile([128, TS], f32, tag="ss")
                nc.sync.dma_start(out=ss_n, in_=ssd[:, t1:t1 + TS])
                ve_n = tab_p.tile([128, TPS, HD], f32, tag="ve")
                nc.sync.dma_start(
                    out=ve_n,
                    in_=ved[t1:t1 + TS, :].rearrange("(tt p) h -> p tt h", p=128))
                xss[m + 1] = xs_n
                tabs[m + 1] = (cc_n, ss_n, ve_n)

            # ---- v projection + transpose to natural + gate-add ----
            ps_v = ps_qkv.tile([128, TS], f32, tag="qkv")
            for c in range(NCK):
                nc.tensor.matmul(ps_v, wv_sb[:, c, :], xs[:, c, :],
                                 start=(c == 0), stop=(c == NCK - 1))
            vT_s = work_p.tile([128, TS], f32, tag="vt")
            nc.scalar.activation(vT_s, ps_v, AF.Copy)
            ps_t = ps_qkv.tile([128, TS], f32, tag="qkv")
            for tt in range(TPS):
                nc.tensor.transpose(ps_t[:, tt * 128:(tt + 1) * 128],
                                    vT_s[:, tt * 128:(tt + 1) * 128], ident)
            for tt in range(TPS):
                nc.vector.scalar_tensor_tensor(
                    out=vn_sb[:, m * TPS + tt, :],
                    in0=ve_sl[:, tt, :], scalar=gate_c[:, tt:tt + 1],
                    in1=ps_t[:, tt * 128:(tt + 1) * 128],
                    op0=OP.mult, op1=OP.add)

            # ---- attention subtiles (+ interleaved c_proj of previous one) ----
            for j in range(TPS):
                t = m * TPS + j
                nlo = max(0, t - NW)
                # Order: one full tile opens the PSUM groups (short dep chain),
                # the masked boundary tiles (diag/edge) go next so their longer
                # exp->mask chains overlap the remaining full tiles' work.
                fulls = [n for n in range(nlo, t + 1)
                         if n != t and n != t - NW]
                ns = []
                if fulls:
                    ns.append(fulls[0])
                ns.append(t)                      # diag (btri)
                if t - NW >= 0:
                    ns.append(t - NW)             # edge (etri)
                ns.extend(fulls[1:])
                L = len(ns)
                last = L - 1
                # hybrid denominator: the first `early` tiles are summed on
                # Pool (their reduces finish well before the subtile ends);
                # the last 3 tiles keep PE ones-matmuls so the chain stays
                # short. sel-matmuls fold the Pool partials into ps_sum.
                early = L - 3 if L >= 6 else 0
                G = (early + 2) // 3
                gsz = [min(3, early - 3 * g) for g in range(G)]
                rows = rows_ab[(m * TPS + j) % 2]
                psy = ps_y.tile([128, HPC * 128], f32, tag="py")
                ps_sum = ps_row.tile([1, TS], f32, tag="rows")
                esg = []
                for _g in range(G):
                    es3 = es3_p.tile([128, 3, HPC * 128], f32r, tag="es3")
                    esg.append(es3)
                for idx, n in enumerate(ns):
                    pss = ps_s.tile([128, HPC * 128], f32, tag="s")
                    nc.tensor.matmul(pss, kT_sb[:, n * 128:(n + 1) * 128],
                                     q4[:, :, j * 128:(j + 1) * 128],
                                     start=True, stop=True)
                    if idx < early:
                        g, gi = idx // 3, idx % 3
                        es = esg[g][:, gi, :]
                    else:
                        es = es_p.tile([128, HPC * 128], f32r, tag="es")
                    nc.scalar.activation(es, pss, AF.Exp)
                    if n == t:
                        nc.vector.tensor_mul(es, es.bitcast(f32), btri_sb)
                    if n == t - NW:
                        nc.vector.tensor_mul(es, es.bitcast(f32), etri_sb)
                    if idx >= early:
                        nc.tensor.matmul(ps_sum, ones_sb, es,
                                         start=(idx == early),
                                         stop=(G == 0 and idx == last))
                    nc.tensor.matmul(psy, vn_sb[:, n, :], es,
                                     start=(idx == 0), stop=(idx == last))
                    if idx < early and (idx % 3 == gsz[idx // 3] - 1):
                        g = idx // 3
                        with nc.allow_low_precision(
                                reason="f32r rows: softmax denominator "
                                       "tolerates TF32-width rounding"):
                            nc.gpsimd.tensor_reduce(
                                rows[32 * g:32 * g + 1, 0:gsz[g], :],
                                esg[g][:, 0:gsz[g], :].bitcast(f32),
                                axis=mybir.AxisListType.C, op=OP.add)
                if G:
                    for b in range(gsz[0]):
                        cnt = sum(1 for x in gsz if x > b)
                        nc.tensor.matmul(ps_sum, sel_sb[:, cnt - 1:cnt],
                                         rows[:, b, :], start=False,
                                         stop=(b == gsz[0] - 1))
                rsum = row_p.tile([1, TS], f32, tag="rsum")
                nc.vector.reciprocal(rsum, ps_sum)
                sbc = bc_p.tile([128, TS], f32, tag="sbc")
                nc.gpsimd.partition_broadcast(sbc, rsum)
                yt4 = yt_p.tile([128, HPC, 128], bf16, tag="yt")
                nc.vector.tensor_mul(
                    yt4.rearrange("p h t -> p (h t)"), psy, sbc)
                yt_tiles[(m, j)] = yt4
                if j > 0:
                    emit_cproj(m, j - 1)

        emit_cproj(NSL - 1, TPS - 1)

    # Restrict the activation-table picker to the one set containing every
    # ACT function we use (exp, ln, square, copy): without this the greedy
    # picker alternates tables, inserting a ~1.3us table load per switch.
    import concourse.hw_specs as hw_specs
    import concourse.bacc as bacc_mod

    orig = hw_specs.get_activation_tables

    def only_combined(arch):
        t = orig(arch)
        return {k: (v if k == "natural_log_exp_and_others" else set())
                for k, v in t.items()}

    hw_specs.get_activation_tables = only_combined
    bacc_mod.get_activation_tables = only_combined
    try:
        nc.compile()
    finally:
        hw_specs.get_activation_tables = orig
        bacc_mod.get_activation_tables = orig
    return nc


def _prep_inputs(x, ve, cos, sin, Wq, Wk, Wv, Wproj, Wgate, W):
    import ml_dtypes

    bf = ml_dtypes.bfloat16
    cosT = np.ascontiguousarray(cos[0, :, 0, :].T)  # (64, T)
    sinT = np.ascontiguousarray(sin[0, :, 0, :].T)
    cc = np.concatenate([cosT, cosT], axis=0).astype(np.float32)
    ss = np.concatenate([sinT, -sinT], axis=0).astype(np.float32)
    p = np.arange(128)[:, None]
    f = np.arange(128)[None, :]
    btri = (p <= f).astype(np.float32)
    etri = (f <= p + (W % 128)).astype(np.float32)
    btri4 = np.ascontiguousarray(np.tile(btri, (1, HPC)))
    etri4 = np.ascontiguousarray(np.tile(etri, (1, HPC)))
    # half-swap permutation: out[p] = in[(p+64) % 128]
    perm = np.zeros((128, 128), dtype=np.float32)
    perm[(np.arange(128) + 64) % 128, np.arange(128)] = 1.0
    sel = np.zeros((128, 3), dtype=np.float32)
    for c in range(3):
        sel[[32 * g for g in range(c + 1)], c] = 1.0

    in_maps = []
    for core in range(8):
        b, g = core // NKV, core % NKV
        hs = slice(g * HPC * HD, (g + 1) * HPC * HD)
        ks = slice(g * HD, (g + 1) * HD)
        in_maps.append({
            "xT": np.ascontiguousarray(x[b].T).astype(bf),
            "wqT": np.ascontiguousarray(Wq[hs, :].T).astype(bf),
            "wkT": np.ascontiguousarray(Wk[ks, :].T).astype(bf),
            "wvT": np.ascontiguousarray(Wv[ks, :].T).astype(bf),
            "wpT": np.ascontiguousarray(Wproj[:, hs].T).astype(bf),
            "wg": np.ascontiguousarray(Wgate[g][:, None]).astype(bf),
            "cc": cc,
            "ss": ss,
            "ve": np.ascontiguousarray(ve[b][:, ks]),
            "btri4": btri4,
            "etri4": etri4,
            "perm": perm,
            "sel": sel.astype(bf),
        })
    return in_maps


def _run(inputs, trace=False):
    from concourse.bass_utils import run_bass_kernel_spmd

    x = np.asarray(inputs["x"], dtype=np.float32)
    ve = np.asarray(inputs["ve"], dtype=np.float32)
    cos = np.asarray(inputs["cos"], dtype=np.float32)
    sin = np.asarray(inputs["sin"], dtype=np.float32)
    Wq = np.asarray(inputs["Wq"], dtype=np.float32)
    Wk = np.asarray(inputs["Wk"], dtype=np.float32)
    Wv = np.asarray(inputs["Wv"], dtype=np.float32)
    Wproj = np.asarray(inputs["Wproj"], dtype=np.float32)
    Wgate = np.asarray(inputs["Wgate"], dtype=np.float32)
    W = int(inputs["window_size"])

    if W not in _compiled:
        _compiled[W] = _build(W)
    nc = _compiled[W]

    in_maps = _prep_inputs(x, ve, cos, sin, Wq, Wk, Wv, Wproj, Wgate, W)
    res = run_bass_kernel_spmd(nc, in_maps, core_ids=list(range(8)), trace=trace)

    out = np.zeros((B, T, C), dtype=np.float32)
    for core in range(8):
        b = core // NKV
        out[b] += res.results[core]["outT"].T
    return out, res


def kernel(**inputs):
    out, _ = _run(inputs, trace=False)
    return out
